# revision 4
# baseline (speedup 1.0000x reference)
"""Trainium2 Bass kernel for nn_NeuralCDE_84189948936255.

Neural CDE with ReversibleHeun solver: B=128 samples, T=1024 time steps,
D=32 data dims, H=64 hidden, W=128 MLP width, C=D+1=33 control dims.

Strategy (data-parallel over batch, 8 cores x 16 samples):
  - The T-1=1023-step scan is inherently sequential; each core runs the
    full scan for its 16 samples, fully unrolled.
  - Leapfrog restructure: expanding ReversibleHeun's updates,
        yhat_{k+2} = yhat_k + v_{k+1}·(dx_k + dx_{k+1})
        y_T        = y_0 + p_0 + Σ_k e_k,   e_k = v_{k+1}·(dx_k+dx_{k+1})/2
    so the scan needs ONE dot product per step against a host-precomputed
    pair-sum table, y drops off the critical path entirely (a lazy
    accumulator), and v is consumed in the same iteration it is produced.
  - All matmul data paths are fp32 (the CDE dynamics amplify per-step
    noise ~1000x; fp32r/bf16 in the main GEMM fails 2e-2) EXCEPT the
    constant vb2 bias injection, which uses an exact bf16 one-hot
    stationary with bf16 hi+lo bias rows (error ~2^-17, harmless).
  - mm1 is split into an early matmul on yhat_{k-1} (off critical path)
    plus an accumulating matmul on 2·W0ᵀ·eT_{k-1}, so the chain goes
    reduce→transpose→matmul without waiting for the yhat state update.
  - mm3/tanh/dot are split into two 264-wide halves so tanh+dot of half 0
    overlap the PE stream of half 1.
  - Device layouts (unchanged from the spread/sample scheme):
      state yhat:  [128, 16] "spread": partition 32j+hl <-> h=16j+hl (hl<16)
      v/w:         [128, 528]: partition 32j+b <-> sample b, free 33*hl+c
      mm3 column-tiled 4x over PE col groups j via tile_position.

kernel(**inputs) takes the FULL unsharded inputs and returns [B,1] fp32.
"""

import os
import sys
import numpy as np

sys.path.insert(0, "/opt/trn_rl_repo")

B, T, D, H, W = 128, 1024, 32, 64, 128
C = D + 1            # 33
O = H * C            # 2112
NCORES = 8
BPC = B // NCORES    # 16 samples per core
NSTEP = int(os.environ.get("NCDE_NSTEP", T - 1))   # 1023 (overridable)

_CACHE = {}


def _silu(x):
    return x / (1.0 + np.exp(-x))


def _host_precompute(ts, ys, iW0, ib0, iW1, ib1, iW2, ib2,
                     vW0, vb0, vW1, vb1, vW2, vb2, rW, rb):
    """All host-side math in fp32; returns per-core input maps + readout fn."""
    import ml_dtypes
    bf16 = ml_dtypes.bfloat16

    ts = np.asarray(ts, np.float32)
    ys = np.asarray(ys, np.float32)

    # control path xs = [t, y] and its increments
    xs = np.concatenate(
        [np.broadcast_to(ts[None, :, None], (B, T, 1)), ys], axis=2
    ).astype(np.float32)                       # [B,T,C]
    dxs = xs[:, 1:] - xs[:, :-1]               # [B,T-1,C] (dx[b,t]=x(t+1)-x(t))

    # initial MLP (runs once -> host)
    x0 = xs[:, 0]
    z = np.maximum(x0 @ iW0.T + ib0, 0.0)
    z = np.maximum(z @ iW1.T + ib1, 0.0)
    y0 = (z @ iW2.T + ib2).astype(np.float32)  # [B,H]

    # folded weights: lipswish = 0.909*silu; fold 0.909 into next layer
    W1f = (0.909 * vW1).astype(np.float32)
    W2f = (0.909 * vW2).astype(np.float32)

    # initial vector field v0 = vf(ts[0], y0)  [B,H,C]
    inp = np.concatenate([np.full((B, 1), ts[0], np.float32), y0], axis=1)
    zz = _silu(inp @ vW0.T + vb0)
    zz = _silu(zz @ W1f.T + vb1)
    v0 = np.tanh(zz @ W2f.T + vb2).reshape(B, H, C).astype(np.float32)

    # p0 = v0 · dx0 / 2 ; yh1 = y0 + v0·dx0
    u0 = np.einsum('bhc,bc->bh', v0, dxs[:, 0]).astype(np.float32)
    p0 = (0.5 * u0).astype(np.float32)          # [B,H]
    yh1 = (y0 + u0).astype(np.float32)          # [B,H]

    # ---- device layouts (shared across cores except dxs2 / yh) ----
    # W0sp: spread-padded vW0[:,1:]^T  [K=128 spread-h, M=128 f]
    W0sp = np.zeros((128, 128), np.float32)
    for j in range(4):
        for hl in range(16):
            W0sp[32 * j + hl, :] = vW0[:, 1 + 16 * j + hl]
    W0dbl = (2.0 * W0sp).astype(np.float32)
    W1T = np.ascontiguousarray(W1f.T)          # [128,128]

    # hc chunk order: group j covers h in [16j,16j+16); free sn -> hl=sn//33,
    # c=sn%33 (s-half 0: hl 0..8, s-half 1: hl 8..16)
    hc_of = np.zeros((4, 528), np.int64)
    for j in range(4):
        sn = np.arange(528)
        hl = sn // 33
        c = sn % 33
        hc_of[j] = (16 * j + hl) * C + c
    W2mov = np.zeros((128, 4 * 528), np.float32)   # [K=128, 2112]
    for j in range(4):
        W2mov[:, 528 * j:528 * (j + 1)] = W2f[hc_of[j], :].T
    vb2mov = np.zeros((4, 528), np.float32)        # rows j = vb2 over chunk j
    for j in range(4):
        vb2mov[j] = vb2[hc_of[j]]
    # bf16 hi/lo split of the bias, one-hot K=8 stationary (exact 0/1)
    vb_hi = vb2mov.astype(bf16)
    vb_lo = (vb2mov - vb_hi.astype(np.float32)).astype(bf16)
    vbcat = np.concatenate([vb_hi, vb_lo], axis=0)          # [8, 528] bf16
    onehot8 = np.zeros((8, 128), np.float32)
    for j in range(4):
        onehot8[j, 32 * j:32 * j + BPC] = 1.0
        onehot8[4 + j, 32 * j:32 * j + BPC] = 1.0
    onehot8 = onehot8.astype(bf16)

    # per-step bias for layer 1: bias0t[:, k] = vb0 + vW0[:,0]*ts[k+1]
    bias0t = (vb0[:, None] + vW0[:, 0][:, None] * ts[None, 1:1 + NSTEP]
              ).astype(np.float32)               # [128, NSTEP]
    vb1c = np.ascontiguousarray(vb1[:, None].astype(np.float32))  # [128,1]

    # pair-sum dx table: dxs2[k] = (dx_k + dx_{k+1})/2, last = dx_last/2
    dpair = 0.5 * dxs[:, :NSTEP, :].copy()       # [B, NSTEP, C]
    dpair[:, :NSTEP - 1, :] += 0.5 * dxs[:, 1:NSTEP, :]

    # ---- per-core tensors ----
    in_maps = []
    for core in range(NCORES):
        bsl = slice(BPC * core, BPC * (core + 1))
        # dxs2 [128, NSTEP*33]: partition 32j+b (b<16) -> dpair[b, k, c]
        dxh = np.zeros((128, NSTEP * C), np.float32)
        flat = dpair[bsl].reshape(BPC, NSTEP * C)
        for j in range(4):
            dxh[32 * j:32 * j + BPC, :] = flat
        # yh0/yh1 spread [128,16]: partition 32j+hl = yh[b, 16j+hl]
        def spread(mat):
            out = np.zeros((128, BPC), np.float32)
            mc = mat[bsl]
            for j in range(4):
                out[32 * j:32 * j + 16, :] = mc[:, 16 * j:16 * j + 16].T
            return out
        in_maps.append(dict(
            W0sp=W0sp, W0dbl=W0dbl, W1T=W1T, W2mov=W2mov,
            vbcat=vbcat, onehot8=onehot8,
            bias0t=bias0t, vb1c=vb1c, dxs2=dxh,
            yh0sp=spread(y0), yh1sp=spread(yh1),
        ))

    def readout(saccs):
        # saccs: list of [128, 16] sample-layout e-accumulators per core
        yT = np.zeros((B, H), np.float32)
        for core in range(NCORES):
            o = saccs[core]
            for j in range(4):
                yT[BPC * core:BPC * (core + 1), 16 * j:16 * j + 16] = \
                    o[32 * j:32 * j + BPC, :]
        yT = yT + y0 + p0
        return (yT @ np.asarray(rW, np.float32).T
                + np.asarray(rb, np.float32)).astype(np.float32)

    return in_maps, readout


def _build_program(nstep):
    import concourse.bass as bass
    import concourse.mybir as mybir
    import concourse.tile as tile
    from concourse import bacc

    f32 = mybir.dt.float32
    bf16 = mybir.dt.bfloat16
    AF = mybir.ActivationFunctionType
    ALU = mybir.AluOpType

    nc = bacc.Bacc("TRN2", target_bir_lowering=False, debug=False)

    dW0sp = nc.dram_tensor("W0sp", [128, 128], f32, kind="ExternalInput")
    dW0dbl = nc.dram_tensor("W0dbl", [128, 128], f32, kind="ExternalInput")
    dW1T = nc.dram_tensor("W1T", [128, 128], f32, kind="ExternalInput")
    dW2mov = nc.dram_tensor("W2mov", [128, 2112], f32, kind="ExternalInput")
    dvbcat = nc.dram_tensor("vbcat", [8, 528], bf16, kind="ExternalInput")
    donehot = nc.dram_tensor("onehot8", [8, 128], bf16, kind="ExternalInput")
    dbias0 = nc.dram_tensor("bias0t", [128, nstep], f32, kind="ExternalInput")
    dvb1 = nc.dram_tensor("vb1c", [128, 1], f32, kind="ExternalInput")
    ddxs2 = nc.dram_tensor("dxs2", [128, nstep * C], f32, kind="ExternalInput")
    dyh0 = nc.dram_tensor("yh0sp", [128, BPC], f32, kind="ExternalInput")
    dyh1 = nc.dram_tensor("yh1sp", [128, BPC], f32, kind="ExternalInput")
    dsout = nc.dram_tensor("sout", [128, BPC], f32, kind="ExternalOutput")

    with tile.TileContext(nc) as tc:
        # ---- persistent SBUF ----
        sW0 = nc.alloc_sbuf_tensor("sW0", [128, 128], f32)
        sW0d = nc.alloc_sbuf_tensor("sW0d", [128, 128], f32)
        sW1 = nc.alloc_sbuf_tensor("sW1", [128, 128], f32)
        sW2 = nc.alloc_sbuf_tensor("sW2", [128, 2112], f32)
        svbc = nc.alloc_sbuf_tensor("svbc", [8, 528], bf16)
        soneh = nc.alloc_sbuf_tensor("soneh", [8, 128], bf16)
        sbias0 = nc.alloc_sbuf_tensor("sbias0", [128, nstep], f32)
        svb1 = nc.alloc_sbuf_tensor("svb1", [128, 1], f32)
        sdx = nc.alloc_sbuf_tensor("sdx", [128, nstep * C], f32)
        syhA = nc.alloc_sbuf_tensor("syhA", [128, BPC], f32)   # yh even
        syhB = nc.alloc_sbuf_tensor("syhB", [128, BPC], f32)   # yh odd
        sv = nc.alloc_sbuf_tensor("sv", [128, 528], f32)       # v (one iter)
        sw = nc.alloc_sbuf_tensor("sw", [128, 528], f32)       # products
        seb = nc.alloc_sbuf_tensor("seb", [128, 32], f32)      # e, cols16:32=0
        seT = nc.alloc_sbuf_tensor("seT", [128, 32], f32)      # transposed e
        sacc = nc.alloc_sbuf_tensor("sacc", [128, BPC], f32)   # sum of e_k
        sz1 = nc.alloc_sbuf_tensor("sz1", [128, BPC], f32)
        sz2 = nc.alloc_sbuf_tensor("sz2", [128, BPC], f32)

        psum1 = nc.alloc_psum_tensor("psum1", [128, BPC], f32)
        psum2 = nc.alloc_psum_tensor("psum2", [128, BPC], f32)
        psum3 = nc.alloc_psum_tensor("psum3", [128, 1024], f32)  # 2 banks

        # ---- prologue ----
        nc.sync.dma_start(sW0[:, :], dW0sp[:, :])
        nc.sync.dma_start(sW0d[:, :], dW0dbl[:, :])
        nc.sync.dma_start(sW1[:, :], dW1T[:, :])
        nc.sync.dma_start(sW2[:, :], dW2mov[:, :])
        nc.sync.dma_start(svbc[:, :], dvbcat[:, :])
        nc.sync.dma_start(soneh[:, :], donehot[:, :])
        nc.sync.dma_start(sbias0[:, :], dbias0[:, :])
        nc.sync.dma_start(svb1[:, :], dvb1[:, :])
        nc.sync.dma_start(sdx[:, :], ddxs2[:, :])
        nc.sync.dma_start(syhA[:, :], dyh0[:, :])
        nc.sync.dma_start(syhB[:, :], dyh1[:, :])
        nc.gpsimd.memset(seb[:, :], 0.0)
        nc.gpsimd.memset(sacc[:, :], 0.0)

        for k in range(nstep):
            # buffer holding yh_{k-1}; updated in this iter to yh_{k+1}
            # (yh_0 lives in syhA, yh_1 in syhB; parity (k-1)%2 selects)
            buf = syhA if (k % 2 == 1) else syhB

            # ---- mm1: psum1 = W0ᵀ·yh_{k+1} (split: yh_{k-1} part early,
            #      2·W0ᵀ·eT_{k-1} part on the chain) ----
            if k == 0:
                # yh_1 directly (host-provided), no eT part
                nc.tensor.matmul(psum1[:, :], sW0[:, :], syhB[:, :],
                                 start=True, stop=True)
            else:
                nc.tensor.matmul(psum1[:, :], sW0[:, :], buf[:, :],
                                 start=True, stop=False)
                nc.tensor.matmul(psum1[:, :], sW0d[:, :], seT[:, 0:BPC],
                                 start=False, stop=True)
                # off-chain state update: yh_{k+1} = yh_{k-1} + 2·eT_{k-1}
                nc.vector.scalar_tensor_tensor(
                    buf[:, :], seT[:, 0:BPC], 2.0, buf[:, :],
                    op0=ALU.mult, op1=ALU.add)

            nc.scalar.activation(sz1[:, :], psum1[:, :], AF.Silu,
                                 bias=sbias0[:, k:k + 1], scale=1.0)
            nc.tensor.matmul(psum2[:, :], sW1[:, :], sz1[:, :],
                             start=True, stop=True)
            nc.scalar.activation(sz2[:, :], psum2[:, :], AF.Silu,
                                 bias=svb1[:, 0:1], scale=1.0)

            # dx pair-sum slice for this iter, broadcast over hl
            dx3 = sdx[:, C * k:C * (k + 1)] \
                .rearrange("p (a n) -> p a n", a=1).broadcast_to([128, 8, C])

            for s in range(2):
                # ---- mm3 half s: bias (one-hot bf16 hi+lo) + 4 col groups ----
                nc.tensor.matmul(psum3[:, 512 * s:512 * s + 264],
                                 soneh[:, :], svbc[:, 264 * s:264 * s + 264],
                                 start=True, stop=False, skip_group_check=True)
                for j in range(4):
                    nc.tensor.matmul(
                        psum3[32 * j:32 * j + 16, 512 * s:512 * s + 264],
                        sz2[:, :],
                        sW2[:, 528 * j + 264 * s:528 * j + 264 * s + 264],
                        start=False, stop=True, skip_group_check=True,
                        tile_position=(0, 32 * j))

                # ---- tanh -> v half; e-dot half ----
                nc.scalar.activation(sv[:, 264 * s:264 * s + 264],
                                     psum3[:, 512 * s:512 * s + 264], AF.Tanh)
                v3 = sv[:, 264 * s:264 * s + 264] \
                    .rearrange("p (s n) -> p s n", n=C)
                w3 = sw[:, 264 * s:264 * s + 264] \
                    .rearrange("p (s n) -> p s n", n=C)
                nc.vector.tensor_tensor(w3, v3, dx3, op=ALU.mult)
                nc.vector.tensor_reduce(seb[:, 8 * s:8 * s + 8], w3,
                                        axis=mybir.AxisListType.X, op=ALU.add)

            # ---- eT, e-accumulation (y path is fully lazy) ----
            nc.vector.transpose(seT[:, :], seb[:, :])
            nc.vector.tensor_tensor(sacc[:, :], sacc[:, :], seb[:, 0:16],
                                    op=ALU.add)

        nc.sync.dma_start(dsout[:, :], sacc[:, :])

    nc.compile()
    return nc


def _get_program(nstep):
    key = nstep
    if key not in _CACHE:
        _CACHE[key] = _build_program(nstep)
    return _CACHE[key]


LAST_EXEC_NS = None


def kernel(**inputs) -> np.ndarray:
    global LAST_EXEC_NS
    in_maps, readout = _host_precompute(**inputs)
    nc = _get_program(NSTEP)
    from concourse.bass_utils import run_bass_kernel_spmd
    kw = {}
    if int(os.environ.get("NCDE_TRACE", "0")):
        kw = dict(trace=True,
                  tmpdir=os.environ.get("NCDE_TRACE_DIR") or None)
    res = run_bass_kernel_spmd(nc, in_maps, core_ids=list(range(NCORES)), **kw)
    LAST_EXEC_NS = res.exec_time_ns
    if res.instructions_and_trace is not None:
        print(f"trace path: {res.instructions_and_trace[1]}", file=sys.stderr)
    saccs = [res.results[c]["sout"] for c in range(NCORES)]
    return readout(saccs)


# revision 11
# speedup vs baseline: 1.0757x; 1.0757x over previous
"""Trainium2 Bass kernel for nn_NeuralCDE_84189948936255.

Neural CDE with ReversibleHeun solver: B=128 samples, T=1024 time steps,
D=32 data dims, H=64 hidden, W=128 MLP width, C=D+1=33 control dims.

Strategy (data-parallel over batch, 8 cores x 16 samples):
  - The T-1=1023-step scan is inherently sequential; each core runs the
    full scan for its 16 samples, fully unrolled.
  - Leapfrog restructure: expanding ReversibleHeun's updates,
        yhat_{k+2} = yhat_k + v_{k+1}·(dx_k + dx_{k+1})
        y_T        = y_0 + p_0 + Σ_k e_k,   e_k = v_{k+1}·(dx_k+dx_{k+1})/2
    so the scan needs ONE dot product per step against a host-precomputed
    pair-sum table, y drops off the critical path entirely (a lazy
    accumulator), and v is consumed in the same iteration it is produced.
  - All matmul data paths are fp32 (the CDE dynamics amplify per-step
    noise ~1000x; fp32r/bf16 in the main GEMM fails 2e-2) EXCEPT the
    constant vb2 bias injection, which uses an exact bf16 one-hot
    stationary with bf16 hi+lo bias rows (error ~2^-17, harmless).
  - mm1 is split into an early matmul on yhat_{k-1} (off critical path)
    plus an accumulating matmul on 2·W0ᵀ·eT_{k-1}, so the chain goes
    reduce→transpose→matmul without waiting for the yhat state update.
  - mm3/tanh/dot are split into two 264-wide halves so tanh+dot of half 0
    overlap the PE stream of half 1.
  - Device layouts (unchanged from the spread/sample scheme):
      state yhat:  [128, 16] "spread": partition 32j+hl <-> h=16j+hl (hl<16)
      v/w:         [128, 528]: partition 32j+b <-> sample b, free 33*hl+c
      mm3 column-tiled 4x over PE col groups j via tile_position.

kernel(**inputs) takes the FULL unsharded inputs and returns [B,1] fp32.
"""

import os
import sys
import numpy as np

sys.path.insert(0, "/opt/trn_rl_repo")

B, T, D, H, W = 128, 1024, 32, 64, 128
C = D + 1            # 33
O = H * C            # 2112
NCORES = 8
BPC = B // NCORES    # 16 samples per core
NSTEP = int(os.environ.get("NCDE_NSTEP", T - 1))   # 1023 (overridable)

_CACHE = {}


def _silu(x):
    return x / (1.0 + np.exp(-x))


def _host_precompute(ts, ys, iW0, ib0, iW1, ib1, iW2, ib2,
                     vW0, vb0, vW1, vb1, vW2, vb2, rW, rb):
    """All host-side math in fp32; returns per-core input maps + readout fn."""
    import ml_dtypes
    bf16 = ml_dtypes.bfloat16

    ts = np.asarray(ts, np.float32)
    ys = np.asarray(ys, np.float32)

    # control path xs = [t, y] and its increments
    xs = np.concatenate(
        [np.broadcast_to(ts[None, :, None], (B, T, 1)), ys], axis=2
    ).astype(np.float32)                       # [B,T,C]
    dxs = xs[:, 1:] - xs[:, :-1]               # [B,T-1,C] (dx[b,t]=x(t+1)-x(t))

    # initial MLP (runs once -> host)
    x0 = xs[:, 0]
    z = np.maximum(x0 @ iW0.T + ib0, 0.0)
    z = np.maximum(z @ iW1.T + ib1, 0.0)
    y0 = (z @ iW2.T + ib2).astype(np.float32)  # [B,H]

    # folded weights: lipswish = 0.909*silu; fold 0.909 into next layer
    W1f = (0.909 * vW1).astype(np.float32)
    W2f = (0.909 * vW2).astype(np.float32)

    # initial vector field v0 = vf(ts[0], y0)  [B,H,C]
    inp = np.concatenate([np.full((B, 1), ts[0], np.float32), y0], axis=1)
    zz = _silu(inp @ vW0.T + vb0)
    zz = _silu(zz @ W1f.T + vb1)
    v0 = np.tanh(zz @ W2f.T + vb2).reshape(B, H, C).astype(np.float32)

    # p0 = v0 · dx0 / 2 ; yh1 = y0 + v0·dx0
    u0 = np.einsum('bhc,bc->bh', v0, dxs[:, 0]).astype(np.float32)
    p0 = (0.5 * u0).astype(np.float32)          # [B,H]
    yh1 = (y0 + u0).astype(np.float32)          # [B,H]

    # ---- device layouts (shared across cores except dxs2 / yh) ----
    # W0sp: spread-padded vW0[:,1:]^T  [K=128 spread-h, M=128 f]
    W0sp = np.zeros((128, 128), np.float32)
    for j in range(4):
        for hl in range(16):
            W0sp[32 * j + hl, :] = vW0[:, 1 + 16 * j + hl]
    W0dbl = (2.0 * W0sp).astype(np.float32)
    W1T = np.ascontiguousarray(W1f.T)          # [128,128]

    # hc chunk order: group j covers h in [16j,16j+16); free sn -> hl=sn//33,
    # c=sn%33 (s-half 0: hl 0..8, s-half 1: hl 8..16)
    hc_of = np.zeros((4, 528), np.int64)
    for j in range(4):
        sn = np.arange(528)
        hl = sn // 33
        c = sn % 33
        hc_of[j] = (16 * j + hl) * C + c
    W2mov = np.zeros((128, 4 * 528), np.float32)   # [K=128, 2112]
    for j in range(4):
        W2mov[:, 528 * j:528 * (j + 1)] = W2f[hc_of[j], :].T
    vb2mov = np.zeros((4, 528), np.float32)        # rows j = vb2 over chunk j
    for j in range(4):
        vb2mov[j] = vb2[hc_of[j]]
    # bf16 hi/lo split of the bias, one-hot K=8 stationary (exact 0/1)
    vb_hi = vb2mov.astype(bf16)
    vb_lo = (vb2mov - vb_hi.astype(np.float32)).astype(bf16)
    vbcat = np.concatenate([vb_hi, vb_lo], axis=0)          # [8, 528] bf16
    onehot8 = np.zeros((8, 128), np.float32)
    for j in range(4):
        onehot8[j, 32 * j:32 * j + BPC] = 1.0
        onehot8[4 + j, 32 * j:32 * j + BPC] = 1.0
    onehot8 = onehot8.astype(bf16)

    # per-step bias for layer 1: bias0t[:, k] = vb0 + vW0[:,0]*ts[k+1]
    bias0t = (vb0[:, None] + vW0[:, 0][:, None] * ts[None, 1:1 + NSTEP]
              ).astype(np.float32)               # [128, NSTEP]
    vb1c = np.ascontiguousarray(vb1[:, None].astype(np.float32))  # [128,1]

    # pair-sum dx table: dxs2[k] = (dx_k + dx_{k+1})/2, last = dx_last/2
    dpair = 0.5 * dxs[:, :NSTEP, :].copy()       # [B, NSTEP, C]
    dpair[:, :NSTEP - 1, :] += 0.5 * dxs[:, 1:NSTEP, :]

    # ---- per-core tensors ----
    in_maps = []
    for core in range(NCORES):
        bsl = slice(BPC * core, BPC * (core + 1))
        # dxs2 [128, NSTEP*33]: partition 32j+b (b<16) -> dpair[b, k, c]
        dxh = np.zeros((128, NSTEP * C), np.float32)
        flat = dpair[bsl].reshape(BPC, NSTEP * C)
        for j in range(4):
            dxh[32 * j:32 * j + BPC, :] = flat
        # yh0/yh1 spread [128,16]: partition 32j+hl = yh[b, 16j+hl]
        def spread(mat):
            out = np.zeros((128, BPC), np.float32)
            mc = mat[bsl]
            for j in range(4):
                out[32 * j:32 * j + 16, :] = mc[:, 16 * j:16 * j + 16].T
            return out
        in_maps.append(dict(
            W0sp=W0sp, W0dbl=W0dbl, W1T=W1T, W2mov=W2mov,
            vbcat=vbcat, onehot8=onehot8,
            bias0t=bias0t, vb1c=vb1c, dxs2=dxh,
            yh0sp=spread(y0), yh1sp=spread(yh1),
        ))

    def readout(saccs):
        # saccs: list of [128, 16] sample-layout e-accumulators per core
        yT = np.zeros((B, H), np.float32)
        for core in range(NCORES):
            o = saccs[core]
            for j in range(4):
                yT[BPC * core:BPC * (core + 1), 16 * j:16 * j + 16] = \
                    o[32 * j:32 * j + BPC, :]
        yT = yT + y0 + p0
        return (yT @ np.asarray(rW, np.float32).T
                + np.asarray(rb, np.float32)).astype(np.float32)

    return in_maps, readout


def _build_program(nstep):
    import concourse.bass as bass
    import concourse.mybir as mybir
    import concourse.tile as tile
    from concourse import bacc

    f32 = mybir.dt.float32
    bf16 = mybir.dt.bfloat16
    AF = mybir.ActivationFunctionType
    ALU = mybir.AluOpType

    nc = bacc.Bacc("TRN2", target_bir_lowering=False, debug=False)

    dW0sp = nc.dram_tensor("W0sp", [128, 128], f32, kind="ExternalInput")
    dW0dbl = nc.dram_tensor("W0dbl", [128, 128], f32, kind="ExternalInput")
    dW1T = nc.dram_tensor("W1T", [128, 128], f32, kind="ExternalInput")
    dW2mov = nc.dram_tensor("W2mov", [128, 2112], f32, kind="ExternalInput")
    dvbcat = nc.dram_tensor("vbcat", [8, 528], bf16, kind="ExternalInput")
    donehot = nc.dram_tensor("onehot8", [8, 128], bf16, kind="ExternalInput")
    dbias0 = nc.dram_tensor("bias0t", [128, nstep], f32, kind="ExternalInput")
    dvb1 = nc.dram_tensor("vb1c", [128, 1], f32, kind="ExternalInput")
    ddxs2 = nc.dram_tensor("dxs2", [128, nstep * C], f32, kind="ExternalInput")
    dyh0 = nc.dram_tensor("yh0sp", [128, BPC], f32, kind="ExternalInput")
    dyh1 = nc.dram_tensor("yh1sp", [128, BPC], f32, kind="ExternalInput")
    dsout = nc.dram_tensor("sout", [128, BPC], f32, kind="ExternalOutput")

    with tile.TileContext(nc) as tc:
        # ---- persistent SBUF ----
        sW0 = nc.alloc_sbuf_tensor("sW0", [128, 128], f32)
        sW0d = nc.alloc_sbuf_tensor("sW0d", [128, 128], f32)
        sW1 = nc.alloc_sbuf_tensor("sW1", [128, 128], f32)
        sW2 = nc.alloc_sbuf_tensor("sW2", [128, 2112], f32)
        svbc = nc.alloc_sbuf_tensor("svbc", [8, 528], bf16)
        soneh = nc.alloc_sbuf_tensor("soneh", [8, 128], bf16)
        sbias0 = nc.alloc_sbuf_tensor("sbias0", [128, nstep], f32)
        svb1 = nc.alloc_sbuf_tensor("svb1", [128, 1], f32)
        # dx table in 8 chunk tensors so iteration 0 only waits on chunk 0
        # (dependencies are tracked per tensor, not per region)
        NCH = 8
        ch_steps = (nstep + NCH - 1) // NCH
        sdxs = []
        for i in range(NCH):
            n_i = min(ch_steps, nstep - i * ch_steps)
            if n_i <= 0:
                break
            sdxs.append(nc.alloc_sbuf_tensor(f"sdx{i}", [128, n_i * C], f32))
        syhA = nc.alloc_sbuf_tensor("syhA", [128, BPC], f32)   # yh even
        syhB = nc.alloc_sbuf_tensor("syhB", [128, BPC], f32)   # yh odd
        sv = nc.alloc_sbuf_tensor("sv", [128, 528], f32)       # v (one iter)
        sw = nc.alloc_sbuf_tensor("sw", [128, 528], f32)       # products
        seb = nc.alloc_sbuf_tensor("seb", [128, 32], f32)      # e, cols16:32=0
        seT = nc.alloc_sbuf_tensor("seT", [128, 32], f32)      # transposed e
        sacc = nc.alloc_sbuf_tensor("sacc", [128, BPC], f32)   # sum of e_k
        sz1 = nc.alloc_sbuf_tensor("sz1", [128, BPC], f32)
        sz2 = nc.alloc_sbuf_tensor("sz2", [128, BPC], f32)

        psum1 = nc.alloc_psum_tensor("psum1", [128, BPC], f32)
        psum2 = nc.alloc_psum_tensor("psum2", [128, BPC], f32)
        # separate tensors per mm3 half: Tile tracks dependencies at tensor
        # granularity, so one [128,1024] tensor would false-serialize
        # mm3 half 1 behind tanh of half 0
        psum3a = nc.alloc_psum_tensor("psum3a", [128, 264], f32)
        psum3b = nc.alloc_psum_tensor("psum3b", [128, 264], f32)
        # keep-warm scratch: dummy matmuls land here to hold the PE HAM
        # clock gate open across the per-iteration DVE/Act tail
        psumw = nc.alloc_psum_tensor("psumw", [128, 264], f32)

        # ---- prologue ----
        nc.sync.dma_start(sW0[:, :], dW0sp[:, :])
        nc.sync.dma_start(sW0d[:, :], dW0dbl[:, :])
        nc.sync.dma_start(sW1[:, :], dW1T[:, :])
        nc.sync.dma_start(sW2[:, :], dW2mov[:, :])
        nc.sync.dma_start(svbc[:, :], dvbcat[:, :])
        nc.sync.dma_start(soneh[:, :], donehot[:, :])
        nc.sync.dma_start(sbias0[:, :], dbias0[:, :])
        nc.sync.dma_start(svb1[:, :], dvb1[:, :])
        for i, sdx_i in enumerate(sdxs):
            off = i * ch_steps * C
            nc.sync.dma_start(sdx_i[:, :],
                              ddxs2[:, off:off + sdx_i.shape[1]])
        nc.sync.dma_start(syhA[:, :], dyh0[:, :])
        nc.sync.dma_start(syhB[:, :], dyh1[:, :])
        nc.gpsimd.memset(seb[:, :], 0.0)
        nc.gpsimd.memset(sacc[:, :], 0.0)

        for k in range(nstep):
            # buffer holding yh_{k-1}; updated in this iter to yh_{k+1}
            # (yh_0 lives in syhA, yh_1 in syhB; parity (k-1)%2 selects)
            buf = syhA if (k % 2 == 1) else syhB

            # ---- mm1: psum1 = W0ᵀ·yh_{k+1} (split: yh_{k-1} part early,
            #      2·W0ᵀ·eT_{k-1} part on the chain) ----
            if k == 0:
                # yh_1 directly (host-provided), no eT part
                nc.tensor.matmul(psum1[:, :], sW0[:, :], syhB[:, :],
                                 start=True, stop=True)
            else:
                nc.tensor.matmul(psum1[:, :], sW0[:, :], buf[:, :],
                                 start=True, stop=False)
                nc.tensor.matmul(psum1[:, :], sW0d[:, :], seT[:, 0:BPC],
                                 start=False, stop=True)
                # off-chain state update: yh_{k+1} = yh_{k-1} + 2·eT_{k-1}
                nc.vector.scalar_tensor_tensor(
                    buf[:, :], seT[:, 0:BPC], 2.0, buf[:, :],
                    op0=ALU.mult, op1=ALU.add)

            nc.scalar.activation(sz1[:, :], psum1[:, :], AF.Silu,
                                 bias=sbias0[:, k:k + 1], scale=1.0)
            # small keep-warm under silu1
            nc.tensor.matmul(psumw[:, 0:64], soneh[:, :], svbc[:, 0:64],
                             start=True, stop=True, skip_group_check=True)
            nc.tensor.matmul(psum2[:, :], sW1[:, :], sz1[:, :],
                             start=True, stop=True)
            nc.scalar.activation(sz2[:, :], psum2[:, :], AF.Silu,
                                 bias=svb1[:, 0:1], scale=1.0)
            # small keep-warm under silu2
            nc.tensor.matmul(psumw[:, 0:64], soneh[:, :], svbc[:, 64:128],
                             start=True, stop=True, skip_group_check=True)

            # dx pair-sum slice for this iter, broadcast over hl
            ci, co = k // ch_steps, (k % ch_steps) * C
            dx3 = sdxs[ci][:, co:co + C] \
                .rearrange("p (a n) -> p a n", a=1).broadcast_to([128, 8, C])

            for s, ps3 in ((0, psum3a), (1, psum3b)):
                # ---- mm3 half s: bias (one-hot bf16 hi+lo) + 4 col groups ----
                nc.tensor.matmul(ps3[:, :],
                                 soneh[:, :], svbc[:, 264 * s:264 * s + 264],
                                 start=True, stop=False, skip_group_check=True)
                for j in range(4):
                    nc.tensor.matmul(
                        ps3[32 * j:32 * j + 16, :],
                        sz2[:, :],
                        sW2[:, 528 * j + 264 * s:528 * j + 264 * s + 264],
                        start=False, stop=True, skip_group_check=True,
                        tile_position=(0, 32 * j))

            # keep-warm dummies: constant operands, dead psum bank; they fill
            # the PE idle window during tanh/dot so the HAM stays at full rate
            for _ in range(4):
                nc.tensor.matmul(psumw[:, :], soneh[:, :], svbc[:, 0:264],
                                 start=True, stop=True, skip_group_check=True)

            for s, ps3 in ((0, psum3a), (1, psum3b)):
                # ---- tanh -> v half; e-dot half ----
                nc.scalar.activation(sv[:, 264 * s:264 * s + 264],
                                     ps3[:, :], AF.Tanh)
                v3 = sv[:, 264 * s:264 * s + 264] \
                    .rearrange("p (s n) -> p s n", n=C)
                w3 = sw[:, 264 * s:264 * s + 264] \
                    .rearrange("p (s n) -> p s n", n=C)
                nc.vector.tensor_tensor(w3, v3, dx3, op=ALU.mult)
                nc.vector.tensor_reduce(seb[:, 8 * s:8 * s + 8], w3,
                                        axis=mybir.AxisListType.X, op=ALU.add)

            # ---- eT, e-accumulation (y path is fully lazy) ----
            nc.vector.transpose(seT[:, :], seb[:, :])
            nc.vector.tensor_tensor(sacc[:, :], sacc[:, :], seb[:, 0:16],
                                    op=ALU.add)

        nc.sync.dma_start(dsout[:, :], sacc[:, :])

    nc.compile()
    return nc


def _get_program(nstep):
    key = nstep
    if key not in _CACHE:
        _CACHE[key] = _build_program(nstep)
    return _CACHE[key]


LAST_EXEC_NS = None


def kernel(**inputs) -> np.ndarray:
    global LAST_EXEC_NS
    in_maps, readout = _host_precompute(**inputs)
    nc = _get_program(NSTEP)
    from concourse.bass_utils import run_bass_kernel_spmd
    kw = {}
    if int(os.environ.get("NCDE_TRACE", "0")):
        kw = dict(trace=True,
                  tmpdir=os.environ.get("NCDE_TRACE_DIR") or None)
    res = run_bass_kernel_spmd(nc, in_maps, core_ids=list(range(NCORES)), **kw)
    LAST_EXEC_NS = res.exec_time_ns
    if res.instructions_and_trace is not None:
        print(f"trace path: {res.instructions_and_trace[1]}", file=sys.stderr)
    saccs = [res.results[c]["sout"] for c in range(NCORES)]
    return readout(saccs)
